# revision 36
# baseline (speedup 1.0000x reference)
"""Trainium2 Bass kernel for a 6-layer transformer decoder (self+cross attention).

Strategy: data-parallel over batch across 8 NeuronCores. Each core runs the
full decoder on its 8-batch-item shard, with activations kept transposed
[C, B_local*T] in SBUF so every projection is a natural lhsT.T @ rhs matmul
with a 512-wide moving dim. Matmul inputs are bf16 (fp32 PSUM accumulate);
residual stream and layernorm statistics stay fp32.

Scheduling: softmax normalization is applied to the attention OUTPUT (psum)
rather than to the exp() scores, so attn-out matmuls depend only on exp and
the colsum/reciprocal chain runs off the critical path. LayerNorm issue is
woven into the following phase's PE stream, and the last LN of each layer
is pipelined across the layer boundary (next layer's q/k projections issue
first, with double-buffered weight tiles).
"""

import numpy as np
import ml_dtypes

L, H, C, DH, FF = 6, 8, 512, 64, 2048
B, T = 64, 128
EPS = 1e-5
NCORES = 8
BL = B // NCORES          # batch items per core
TB = BL * T               # 1024 activation columns per core
NC = C // 128             # 4 channel chunks
NF = FF // 128            # 16 ff chunks
NPAIR = H // 2            # head pairs
HD = H * DH               # 512
P = 128
NORM = 1.0 / (T * C)      # layernorm 1/N, folded into the stats matmul

_BF = ml_dtypes.bfloat16

_cache = {}


def _build():
    from contextlib import ExitStack

    import concourse.bass as bass  # noqa: F401
    import concourse.tile as tile
    import concourse.mybir as mybir
    from concourse import bacc

    dt = mybir.dt
    AF = mybir.ActivationFunctionType
    OP = mybir.AluOpType
    f32, bf16 = dt.float32, dt.bfloat16

    nc = bacc.Bacc("TRN2", target_bir_lowering=False, debug=False, num_devices=NCORES)

    d_xT = nc.dram_tensor("xT", [C, TB], f32, kind="ExternalInput").ap()
    d_xTb = nc.dram_tensor("xTb", [C, TB], bf16, kind="ExternalInput").ap()
    d_eT = nc.dram_tensor("eT", [C, TB], bf16, kind="ExternalInput").ap()
    d_w = {}
    for name in ("wq", "wk", "wv", "cq", "ck", "cv"):
        d_w[name] = nc.dram_tensor(name, [L, P, NC, HD], bf16, kind="ExternalInput").ap()
    d_w1 = nc.dram_tensor("w1", [L, P, NC, FF], bf16, kind="ExternalInput").ap()
    d_w2 = nc.dram_tensor("w2", [L, P, NF, C], bf16, kind="ExternalInput").ap()
    d_bqk = nc.dram_tensor("bqk", [P, L * 16], f32, kind="ExternalInput").ap()
    d_bvb = nc.dram_tensor("bvb", [L, 2, P, HD], f32, kind="ExternalInput").ap()
    d_b1 = nc.dram_tensor("b1", [P, L * NF], f32, kind="ExternalInput").ap()
    d_b2 = nc.dram_tensor("b2", [P, L * NC], f32, kind="ExternalInput").ap()
    d_out = nc.dram_tensor("outT", [C, TB], f32, kind="ExternalOutput").ap()

    def act_recip(out, in_):
        # ACT-engine reciprocal (~1e-5 rel err measured on hw for this value
        # range); bass's wrapper refuses Reciprocal so emit directly.
        nc.scalar.add_instruction(mybir.InstActivation(
            name=nc.get_next_instruction_name(),
            func=AF.Reciprocal,
            ins=[nc.scalar.lower_ap(in_),
                 mybir.ImmediateValue(dtype=f32, value=0.0),
                 mybir.ImmediateValue(dtype=f32, value=1.0),
                 mybir.ImmediateValue(dtype=f32, value=0.0)],
            outs=[nc.scalar.lower_ap(out)],
        ))

    with tile.TileContext(nc) as tc:
        with ExitStack() as ctx:
            cpool = ctx.enter_context(tc.tile_pool(name="const", bufs=1))
            apool = ctx.enter_context(tc.tile_pool(name="acts", bufs=1))
            wpool = ctx.enter_context(tc.tile_pool(name="wts", bufs=1))
            wdpool = ctx.enter_context(tc.tile_pool(name="wdbl", bufs=2))
            tpool = ctx.enter_context(tc.tile_pool(name="tmp", bufs=2))
            ps_pj = ctx.enter_context(tc.tile_pool(name="pj", bufs=3, space="PSUM"))
            ps_sc = ctx.enter_context(tc.tile_pool(name="sc", bufs=3, space="PSUM"))
            ps_ao = ctx.enter_context(tc.tile_pool(name="ao", bufs=2, space="PSUM"))

            # ---- constants ----
            ones128b = cpool.tile([P, P], bf16, tag="ones128b")
            nc.vector.memset(ones128b, 1.0)
            onesN = cpool.tile([P, P], f32, tag="onesN")
            nc.vector.memset(onesN, NORM)      # ones/65536 for LN stats matmul
            eps_t = cpool.tile([P, 1], f32, tag="eps")
            nc.vector.memset(eps_t, EPS)
            zero_t = cpool.tile([P, 1], f32, tag="zero")
            nc.vector.memset(zero_t, 0.0)
            pre_t = cpool.tile([P, 1], f32, tag="pre")
            magic_t = cpool.tile([P, 4], dt.uint32, tag="magic")
            nc.vector.memset(magic_t, 0x1fbd1df5)  # sqrt-bits initial guess
            bqk_s = cpool.tile([P, L * 16], f32, tag="bqk")
            b1_s = cpool.tile([P, L * NF], f32, tag="b1")
            b2_s = cpool.tile([P, L * NC], f32, tag="b2")

            def act_preload(func):
                # dummy op so the activation-table load happens off-path
                if func == AF.Reciprocal:
                    act_recip(pre_t, eps_t)
                else:
                    nc.scalar.activation(pre_t, eps_t, func, bias=eps_t,
                                         scale=1.0)

            # ---- persistent activations (kc-major merged tiles per half) ----
            xres = [apool.tile([P, NC * 512], f32, tag=f"xres{h}", name=f"xres{h}")
                    for h in range(2)]
            xn = [apool.tile([P, NC * 512], bf16, tag=f"xn{h}", name=f"xn{h}")
                  for h in range(2)]
            eTs = [apool.tile([P, NC * 512], bf16, tag=f"eT{h}", name=f"eT{h}")
                   for h in range(2)]

            def load_io(dst_list, src, k, h):
                rs = slice(k * P, (k + 1) * P)
                cs_ = slice(h * 512, (h + 1) * 512)
                ts_ = slice(k * 512, (k + 1) * 512)
                nc.sync.dma_start(out=dst_list[h][:, ts_], in_=src[rs, cs_])

            def dma_w(wt, name, l):
                # chunked weight DMA: one dma_start per kc so transfers spread
                # across DMA queues (single queue is only ~20 GB/s)
                for kc in range(NC):
                    nc.sync.dma_start(out=wt[:, kc, :], in_=d_w[name][l][:, kc, :])

            def dma_w12(wt, dram, l, n2, n3, inner):
                chunk = n3 // inner
                for kc in range(n2):
                    for i in range(inner):
                        sl = slice(i * chunk, (i + 1) * chunk)
                        nc.sync.dma_start(out=wt[:, kc, sl], in_=dram[l][:, kc, sl])

            def new_wqkv():
                w = {}
                for name in ("wq", "wk", "wv"):
                    w[name] = wdpool.tile([P, NC, HD], bf16, tag=name, name=name)
                    dma_w(w[name], name, new_wqkv.layer)
                for i, name in enumerate(("bvs", "bvc")):
                    w[name] = wdpool.tile([P, HD], f32, tag=name, name=name)
                    nc.sync.dma_start(out=w[name], in_=d_bvb[new_wqkv.layer, i])
                return w

            # q/k/v weights for layer 0 + input activations, ordered so the
            # first projections can start as soon as possible
            new_wqkv.layer = 0
            wqkv_cur = new_wqkv()
            for k in range(NC):
                load_io(xn, d_xTb, k, 0)
            for k in range(NC):
                load_io(xn, d_xTb, k, 1)
            # bias tables are first needed ~20us in; keep them off the
            # startup-critical DMA queues
            nc.sync.dma_start(out=bqk_s, in_=d_bqk)
            nc.sync.dma_start(out=b1_s, in_=d_b1)
            nc.sync.dma_start(out=b2_s, in_=d_b2)
            for k in range(NC):
                for h in range(2):
                    load_io(xres, d_xT, k, h)
            for k in range(NC):
                for h in range(2):
                    load_io(eTs, d_eT, k, h)

            def xsl(kc, bb=None):
                if bb is None:
                    return slice(kc * 512, (kc + 1) * 512)
                return slice(kc * 512 + bb * P, kc * 512 + (bb + 1) * P)

            qT = [[apool.tile([P, 512], bf16, tag=f"qT{k}_{h}", name=f"qT{k}_{h}")
                   for h in range(2)] for k in range(NC)]
            kT = [[apool.tile([P, 512], bf16, tag=f"kT{k}_{h}", name=f"kT{k}_{h}")
                   for h in range(2)] for k in range(NC)]
            kcT = [[apool.tile([P, 512], bf16, tag=f"kcT{k}_{h}", name=f"kcT{k}_{h}")
                    for h in range(2)] for k in range(NC)]
            vS = [apool.tile([P, HD], bf16, tag=f"v{b}", name=f"v{b}") for b in range(BL)]
            vC = [apool.tile([P, HD], bf16, tag=f"vc{b}", name=f"vc{b}") for b in range(BL)]
            hT = [apool.tile([P, 512], bf16, tag=f"hT{f}", name=f"hT{f}") for f in range(NF)]

            def proj_qk(dst, wt, src, mi, l, scale, halves=(0, 1), evsplit=1):
                for h2 in halves:
                    for mc in range(NC):
                        pj = ps_pj.tile([P, 512], f32, tag="pj", name="pj")
                        for kc in range(NC):
                            nc.tensor.matmul(pj, wt[:, kc, mc * P:(mc + 1) * P],
                                             src[h2][:, xsl(kc)],
                                             start=(kc == 0), stop=(kc == NC - 1))
                        col = (l * 4 + mi) * 4 + mc
                        step = 512 // evsplit
                        for e in range(evsplit):
                            sl = slice(e * step, (e + 1) * step)
                            nc.scalar.activation(dst[mc][h2][:, sl], pj[:, sl],
                                                 AF.Identity,
                                                 bias=bqk_s[:, col:col + 1],
                                                 scale=scale)

            def proj_v(dst, wt, src, bias_tile, bs):
                for b in bs:
                    h2, bb = divmod(b, 4)
                    pj = ps_pj.tile([P, 512], f32, tag="pj", name="pj")
                    for kc in range(NC):
                        nc.tensor.matmul(pj, src[h2][:, xsl(kc, bb)],
                                         wt[:, kc, :],
                                         start=(kc == 0), stop=(kc == NC - 1))
                    nc.vector.tensor_tensor(dst[b], pj, bias_tile, op=OP.add)

            def new_st():
                return tpool.tile([P, 4, 5], f32, tag="ST", name="ST", bufs=8)

            def attention_half(kTl, vl, ST, h2, pe_filler=None, post_hook=None):
                # scores + exp per item; attn-out reads RAW exp scores and the
                # softmax 1/colsum is applied to the psum output at eviction,
                # so colsum/recip sit off the scores->attnout critical path.
                expTs = {}
                for bb in range(4):
                    sce = ps_sc.tile([P, 512], f32, tag="sc", name="sc")
                    sco = ps_sc.tile([P, 512], f32, tag="sc", name="sc")
                    for p in range(NPAIR):
                        nc.tensor.matmul(sce[:, p * P:(p + 1) * P],
                                         kTl[p][h2][0:64, bb * P:(bb + 1) * P],
                                         qT[p][h2][0:64, bb * P:(bb + 1) * P],
                                         start=True, stop=True,
                                         tile_position=(0, 0))
                        nc.tensor.matmul(sco[:, p * P:(p + 1) * P],
                                         kTl[p][h2][64:128, bb * P:(bb + 1) * P],
                                         qT[p][h2][64:128, bb * P:(bb + 1) * P],
                                         start=True, stop=True,
                                         tile_position=(64, 0))
                    expT = tpool.tile([P, TB], bf16, tag="expT", name="expT",
                                      bufs=4)
                    nc.scalar.activation(expT[:, 0:512], sce, AF.Exp,
                                         bias=zero_t, scale=1.0)
                    nc.scalar.activation(expT[:, 512:1024], sco, AF.Exp,
                                         bias=zero_t, scale=1.0)
                    expTs[bb] = expT
                # pull the Reciprocal table load off-path now; the Identity
                # evicts issued in fillers live in every table, so only
                # Exp<->Reciprocal transitions exist and both are preloaded
                act_preload(AF.Reciprocal)
                if pe_filler is not None:
                    pe_filler()
                x3 = xres[h2].rearrange("p (k n) -> p k n", k=NC)
                for bb in range(4):
                    b = h2 * 4 + bb
                    ao = ps_ao.tile([P, 512], f32, tag="ao", name="ao")
                    for p in range(NPAIR):
                        for j in range(2):
                            h = 2 * p + j
                            pos = (h % 2) * 512 + (h // 2) * P
                            nc.tensor.matmul(ao[j * 64:(j + 1) * 64, p * P:(p + 1) * P],
                                             vl[b][:, h * 64:(h + 1) * 64],
                                             expTs[bb][:, pos:pos + P],
                                             start=True, stop=True,
                                             tile_position=(0, j * 64))
                    # denominators for this item: colsum over s via M=64
                    # ones-matmuls into partition-halves of ONE psum tile
                    # (column-split tiles run concurrently on the PE), then a
                    # single full-width reciprocal: ddrC[j-rows] = 1/den_j
                    ddrC = tpool.tile([P, 512], bf16, tag="ddr", name="ddr",
                                      bufs=4)
                    dsum = ps_sc.tile([P, 512], f32, tag="sc", name="sc")
                    for j in range(2):
                        sl = slice(j * 512, (j + 1) * 512)
                        nc.tensor.matmul(dsum[j * 64:(j + 1) * 64, :],
                                         ones128b[:, 0:64], expTs[bb][:, sl],
                                         start=True, stop=True,
                                         tile_position=(0, j * 64))
                    act_recip(ddrC, dsum)
                    aot = tpool.tile([P, 512], f32, tag="aot", name="aot",
                                     bufs=4)
                    nc.vector.tensor_tensor(aot, ao, ddrC, op=OP.mult)
                    dst = x3[:, :, bb * P:(bb + 1) * P]
                    nc.vector.scalar_tensor_tensor(
                        dst, aot.rearrange("p (k n) -> p k n", k=NC),
                        0.0, dst, op0=OP.add, op1=OP.add,
                        accum_out=ST[:, bb, 0:1])
                if post_hook is not None:
                    post_hook()

            def rsqrt_dve(rr, v):
                # rr = 1/sqrt(v) DVE-only (no ACT table): accurate reciprocal,
                # sqrt-bits initial guess, 2 Newton steps (~5e-6 rel err)
                r = tpool.tile([P, 4], f32, tag="rs_r", name="rs_r")
                nc.vector.reciprocal(r, v)
                yu = rr.bitcast(dt.uint32)
                nc.vector.tensor_scalar(yu, r.bitcast(dt.uint32), 1, None,
                                        op0=OP.logical_shift_right)
                nc.vector.tensor_tensor(yu, yu, magic_t, op=OP.add)
                t = tpool.tile([P, 4], f32, tag="rs_t", name="rs_t")
                for _ in range(2):
                    nc.vector.tensor_tensor(t, rr, rr, op=OP.mult)
                    nc.vector.tensor_tensor(t, t, v, op=OP.mult)
                    nc.vector.tensor_scalar(t, t, -0.5, 1.5,
                                            op0=OP.mult, op1=OP.add)
                    nc.vector.tensor_tensor(rr, rr, t, op=OP.mult)

            def ln_sq(ST, h2, nsum=1):
                # stats per batch item over (T, C); col 0(-3) summed at evict,
                # one fused square+reduce per item fills col 4. Split across
                # ACT (Square lives in every table) and DVE to halve the
                # serial chain length.
                x3 = xres[h2].rearrange("p (k n) -> p k n", k=NC)
                for bb in range(4):
                    sq = tpool.tile([P, 512], bf16, tag="sq", name="sq")
                    src = x3[:, :, bb * P:(bb + 1) * P]
                    nc.vector.scalar_tensor_tensor(
                        sq.rearrange("p (k n) -> p k n", k=NC), src, 1.0,
                        src, op0=OP.mult, op1=OP.mult,
                        accum_out=ST[:, bb, 4:5])
                if nsum == 1:
                    # cols 1-3 unwritten in attention mode; zero them so the
                    # stats matmul never reads uninitialized sbuf
                    nc.vector.memset(ST[:, :, 1:4], 0.0)

            def ln_fin(ST, h2, nsum=1, last=False, dma_out=False):
                x3 = xres[h2].rearrange("p (k n) -> p k n", k=NC)
                tot = ps_pj.tile([P, 20], f32, tag="pj", name="pj")
                nc.tensor.matmul(tot, onesN,
                                 ST.rearrange("p a b -> p (a b)"),
                                 start=True, stop=True)
                tot3 = tot.rearrange("p (a b) -> p a b", b=5)
                mm_ = tpool.tile([P, 4], f32, tag="mm_", name="mm_")
                if nsum == 1:
                    nc.vector.tensor_copy(out=mm_, in_=tot3[:, :, 0])
                else:
                    nc.vector.reduce_sum(mm_, tot3[:, :, 0:nsum],
                                         axis=mybir.AxisListType.X)
                var = tpool.tile([P, 4], f32, tag="var", name="var")
                nc.vector.tensor_tensor(var, mm_, mm_, op=OP.mult)
                # var = (sqsum + eps) - mu^2, eps folded in
                nc.vector.scalar_tensor_tensor(var, tot3[:, :, 4], eps_t, var,
                                               op0=OP.add, op1=OP.subtract)
                rr = tpool.tile([P, 4], f32, tag="rr", name="rr")
                rsqrt_dve(rr, var)
                xn3 = xn[h2].rearrange("p (k n) -> p k n", k=NC)
                for bb in range(4):
                    src = x3[:, :, bb * P:(bb + 1) * P]
                    if not last:
                        # bf16 normalized copy first — unblocks the next
                        # phase's matmuls before the fp32 in-place update
                        nc.vector.tensor_scalar(xn3[:, :, bb * P:(bb + 1) * P],
                                                src, mm_[:, bb:bb + 1],
                                                rr[:, bb:bb + 1],
                                                op0=OP.subtract, op1=OP.mult)
                for bb in range(4):
                    src = x3[:, :, bb * P:(bb + 1) * P]
                    # fp32 in-place update is off the critical path (next
                    # reader is the following phase's residual add, ~10us
                    # later) — run it on the otherwise-idle GpSimd engine
                    nc.gpsimd.tensor_scalar(src, src, mm_[:, bb:bb + 1],
                                            rr[:, bb:bb + 1],
                                            op0=OP.subtract, op1=OP.mult)
                    if dma_out:
                        for k in range(NC):
                            nc.sync.dma_start(
                                out=d_out[k * P:(k + 1) * P,
                                          h2 * 512 + bb * P:
                                          h2 * 512 + (bb + 1) * P],
                                in_=xres[h2][:, k * 512 + bb * P:
                                             k * 512 + (bb + 1) * P])

            def ln_issue(ST, h2, nsum=1, last=False, dma_out=False):
                ln_sq(ST, h2, nsum)
                ln_fin(ST, h2, nsum, last, dma_out)

            def ffn_w1(h2, w1s, l):
                for fc in range(NF):
                    pj = ps_pj.tile([P, 512], f32, tag="pj", name="pj")
                    for kc in range(NC):
                        nc.tensor.matmul(pj, w1s[:, kc, fc * P:(fc + 1) * P],
                                         xn[h2][:, xsl(kc)],
                                         start=(kc == 0), stop=(kc == NC - 1))
                    col = l * NF + fc
                    nc.scalar.activation(hT[fc], pj, AF.Relu,
                                         bias=b1_s[:, col:col + 1], scale=1.0)

            def ffn_w2(h2, ST, w2s, l, mcs=range(NC)):
                for mc in mcs:
                    pj = ps_pj.tile([P, 512], f32, tag="pj", name="pj")
                    for fc in range(NF):
                        nc.tensor.matmul(pj, w2s[:, fc, mc * P:(mc + 1) * P],
                                         hT[fc],
                                         start=(fc == 0), stop=(fc == NF - 1))
                    b2col = b2_s[:, l * NC + mc:l * NC + mc + 1]
                    for bb in range(4):
                        dst = xres[h2][:, xsl(mc, bb)]
                        nc.vector.scalar_tensor_tensor(
                            dst, pj[:, bb * P:(bb + 1) * P],
                            b2col, dst, op0=OP.add, op1=OP.add,
                            accum_out=ST[:, bb, mc:mc + 1])

            for l in range(L):
                # single-buffered weights for this layer (all consumed
                # mid-layer, so their DMA overlaps the previous layer's tail)
                bvs = wqkv_cur["bvs"]
                bvc = wqkv_cur["bvc"]
                wts = {}
                for name in ("cq", "ck", "cv"):
                    w = wpool.tile([P, NC, HD], bf16, tag=name, name=name)
                    dma_w(w, name, l)
                    wts[name] = w
                w1s = wpool.tile([P, NC, FF], bf16, tag="w1", name="w1")
                dma_w12(w1s, d_w1, l, NC, FF, 4)
                w2s = wpool.tile([P, NF, C], bf16, tag="w2", name="w2")
                dma_w12(w2s, d_w2, l, NF, C, 1)
                # prefetch next layer's q/k/v weights (double-buffered pool)
                if l + 1 < L:
                    new_wqkv.layer = l + 1
                    wqkv_next = new_wqkv()

                # --- SA projections: h0 q/k came from the previous layer's
                # tail (except l=0); v and h1 parts issue here ---
                if l == 0:
                    proj_qk(qT, wqkv_cur["wq"], xn, 0, l, 0.125, halves=(0,))
                    proj_qk(kT, wqkv_cur["wk"], xn, 1, l, 1.0, halves=(0,))
                proj_v(vS, wqkv_cur["wv"], xn, bvs, range(0, 4))
                proj_qk(qT, wqkv_cur["wq"], xn, 0, l, 0.125, halves=(1,))
                proj_qk(kT, wqkv_cur["wk"], xn, 1, l, 1.0, halves=(1,))
                proj_v(vS, wqkv_cur["wv"], xn, bvs, range(4, 8))

                ST1 = [new_st(), new_st()]
                ST2 = [new_st(), new_st()]
                ST3 = [new_st(), new_st()]

                # --- self attention ---
                def fill_sa0():
                    proj_qk(kcT, wts["ck"], eTs, 3, l, 1.0, halves=(0,))
                    proj_v(vC, wts["cv"], eTs, bvc, range(0, 4))

                attention_half(kT, vS, ST1[0], 0, pe_filler=fill_sa0)

                def fill_sa1():
                    # LN1h0 entirely ahead of the cv evicts in the DVE queue;
                    # its stats matmul is covered by ck-h1's projections
                    ln_sq(ST1[0], 0)
                    proj_qk(kcT, wts["ck"], eTs, 3, l, 1.0, halves=(1,))
                    ln_fin(ST1[0], 0)
                    proj_v(vC, wts["cv"], eTs, bvc, range(4, 8))

                def post_sa1():
                    proj_qk(qT, wts["cq"], xn, 2, l, 0.125, halves=(0,),
                            evsplit=2)
                    act_preload(AF.Exp)

                attention_half(kT, vS, ST1[1], 1,
                               pe_filler=fill_sa1, post_hook=post_sa1)

                # --- cross attention ---
                def fill_ca0():
                    ln_issue(ST1[1], 1)

                def post_ca0():
                    proj_qk(qT, wts["cq"], xn, 2, l, 0.125, halves=(1,),
                            evsplit=2)
                    ln_issue(ST2[0], 0)
                    act_preload(AF.Exp)

                attention_half(kcT, vC, ST2[0], 0,
                               pe_filler=fill_ca0, post_hook=post_ca0)

                def fill_ca1():
                    # FFN h0 fills CA1's ACT-bound window (xn[0] ready: LN2h0
                    # was issued in post_ca0)
                    ffn_w1(0, w1s, l)

                attention_half(kcT, vC, ST2[1], 1, pe_filler=fill_ca1)

                # --- feed-forward ---
                ln_sq(ST2[1], 1)
                ffn_w2(0, ST3[0], w2s, l, mcs=(0,))
                ln_fin(ST2[1], 1)
                ffn_w2(0, ST3[0], w2s, l, mcs=(1, 2, 3))
                ffn_w1(1, w1s, l)
                ln_issue(ST3[0], 0, nsum=4, last=(l == L - 1),
                         dma_out=(l == L - 1))
                ffn_w2(1, ST3[1], w2s, l)

                # --- layer tail: pipeline next layer's h0 q/k projections
                # around the final LN of this layer ---
                if l + 1 < L:
                    proj_qk(qT, wqkv_next["wq"], xn, 0, l + 1, 0.125,
                            halves=(0,))
                    ln_issue(ST3[1], 1, nsum=4)
                    proj_qk(kT, wqkv_next["wk"], xn, 1, l + 1, 1.0,
                            halves=(0,))
                    wqkv_cur = wqkv_next
                else:
                    ln_issue(ST3[1], 1, nsum=4, last=True, dma_out=True)

    nc.compile()
    return nc


def _prep_shared(inputs):
    """Host-side weight repacking (shared across cores)."""
    def packw(w):  # [L,H,C,DH] -> [L,128,NC,H*DH]  (c = kc*128+p)
        w2 = np.ascontiguousarray(w.transpose(0, 2, 1, 3)).reshape(L, C, HD)
        return np.ascontiguousarray(
            w2.reshape(L, NC, P, HD).transpose(0, 2, 1, 3)).astype(_BF)

    shared = {}
    for nm, key in (("wq", "sa_wq"), ("wk", "sa_wk"), ("wv", "sa_wv"),
                    ("cq", "ca_wq"), ("ck", "ca_wk"), ("cv", "ca_wv")):
        shared[nm] = packw(inputs[key])
    shared["w1"] = np.ascontiguousarray(
        inputs["ff_w1"].reshape(L, NC, P, FF).transpose(0, 2, 1, 3)).astype(_BF)
    shared["w2"] = np.ascontiguousarray(
        inputs["ff_w2"].reshape(L, NF, P, C).transpose(0, 2, 1, 3)).astype(_BF)

    bqk = np.zeros((P, L * 16), np.float32)
    for l in range(L):
        for mi, (bias, s) in enumerate((
                (inputs["sa_bq"][l], 0.125), (inputs["sa_bk"][l], 1.0),
                (inputs["ca_bq"][l], 0.125), (inputs["ca_bk"][l], 1.0))):
            flat = bias.reshape(HD).astype(np.float32) * s
            for mc in range(NC):
                bqk[:, (l * 4 + mi) * 4 + mc] = flat[mc * P:(mc + 1) * P]
    shared["bqk"] = bqk

    bv = np.stack([inputs["sa_bv"].reshape(L, HD),
                   inputs["ca_bv"].reshape(L, HD)], axis=1).astype(np.float32)
    shared["bvb"] = np.ascontiguousarray(
        np.broadcast_to(bv[:, :, None, :], (L, 2, P, HD)))

    b1 = np.zeros((P, L * NF), np.float32)
    for l in range(L):
        for fc in range(NF):
            b1[:, l * NF + fc] = inputs["ff_b1"][l, fc * P:(fc + 1) * P]
    shared["b1"] = b1
    b2 = np.zeros((P, L * NC), np.float32)
    for l in range(L):
        for mc in range(NC):
            b2[:, l * NC + mc] = inputs["ff_b2"][l, mc * P:(mc + 1) * P]
    shared["b2"] = b2
    return shared


LAST_RESULT = None


def _install_ntff_hook():
    """Register the axon NTFF profile hook that the image's antenv lacks.

    Only used for local benchmarking (KERNEL_TRACE=1); inert otherwise.
    """
    import sys
    import types
    try:
        import antenv
        if getattr(antenv, "axon_hooks", None) is not None:
            return
        from trn_agent_boot.trn_boot import _ntff_profile_via_ctypes
        mod = types.ModuleType("antenv.axon_hooks")
        mod._hook = _ntff_profile_via_ctypes("/opt/axon/libaxon_pjrt.so")

        def get_axon_ntff_profile_hook():
            return mod._hook

        def set_axon_ntff_profile_hook(h):
            mod._hook = h

        mod.get_axon_ntff_profile_hook = get_axon_ntff_profile_hook
        mod.set_axon_ntff_profile_hook = set_axon_ntff_profile_hook
        sys.modules["antenv.axon_hooks"] = mod
        antenv.axon_hooks = mod
    except Exception as e:  # pragma: no cover - profiling is best-effort
        print(f"ntff hook install failed: {e}")


def kernel(**inputs):
    global LAST_RESULT
    import os
    inputs = {k: np.asarray(v) for k, v in inputs.items()}
    if "nc" not in _cache:
        _cache["nc"] = _build()
    nc = _cache["nc"]

    shared = _prep_shared(inputs)
    x = inputs["x"].astype(np.float32)
    enc = inputs["encoder_output"].astype(np.float32)

    in_maps = []
    for core in range(NCORES):
        sl = slice(core * BL, (core + 1) * BL)
        xT = np.ascontiguousarray(x[sl].transpose(2, 0, 1)).reshape(C, TB)
        eT = np.ascontiguousarray(enc[sl].transpose(2, 0, 1)).reshape(C, TB)
        m = dict(shared)
        m["xT"] = xT
        m["xTb"] = xT.astype(_BF)
        m["eT"] = eT.astype(_BF)
        in_maps.append(m)

    trace = bool(int(os.environ.get("KERNEL_TRACE", "0")))
    if trace:
        _install_ntff_hook()
    from concourse.bass_utils import run_bass_kernel_spmd
    res = run_bass_kernel_spmd(nc, in_maps, list(range(NCORES)), trace=trace,
                               trace_cores=[0])
    LAST_RESULT = res

    out = np.empty((B, T, C), np.float32)
    for core in range(NCORES):
        outT = res.results[core]["outT"]  # [C, TB]
        out[core * BL:(core + 1) * BL] = outT.reshape(C, BL, T).transpose(1, 2, 0)
    return out


# revision 37
# speedup vs baseline: 1.5744x; 1.5744x over previous
"""Trainium2 Bass kernel for a 6-layer transformer decoder (self+cross attention).

Strategy: data-parallel over batch across 8 NeuronCores. Each core runs the
full decoder on its 8-batch-item shard, with activations kept transposed
[C, B_local*T] in SBUF so every projection is a natural lhsT.T @ rhs matmul
with a 512-wide moving dim. Matmul inputs are bf16 (fp32 PSUM accumulate);
residual stream and layernorm statistics stay fp32.

Scheduling: softmax normalization is applied to the attention OUTPUT (psum)
rather than to the exp() scores, so attn-out matmuls depend only on exp and
the colsum/reciprocal chain runs off the critical path. LayerNorm issue is
woven into the following phase's PE stream, and the last LN of each layer
is pipelined across the layer boundary (next layer's q/k projections issue
first, with double-buffered weight tiles).
"""

import numpy as np
import ml_dtypes

L, H, C, DH, FF = 6, 8, 512, 64, 2048
B, T = 64, 128
EPS = 1e-5
NCORES = 8
BL = B // NCORES          # batch items per core
TB = BL * T               # 1024 activation columns per core
NC = C // 128             # 4 channel chunks
NF = FF // 128            # 16 ff chunks
NPAIR = H // 2            # head pairs
HD = H * DH               # 512
P = 128
NORM = 1.0 / (T * C)      # layernorm 1/N, folded into the stats matmul

_BF = ml_dtypes.bfloat16

_cache = {}


def _build():
    from contextlib import ExitStack

    import concourse.bass as bass  # noqa: F401
    import concourse.tile as tile
    import concourse.mybir as mybir
    from concourse import bacc

    dt = mybir.dt
    AF = mybir.ActivationFunctionType
    OP = mybir.AluOpType
    f32, bf16 = dt.float32, dt.bfloat16

    nc = bacc.Bacc("TRN2", target_bir_lowering=False, debug=False, num_devices=NCORES)

    d_xT = nc.dram_tensor("xT", [C, TB], f32, kind="ExternalInput").ap()
    d_xTb = nc.dram_tensor("xTb", [C, TB], bf16, kind="ExternalInput").ap()
    d_eT = nc.dram_tensor("eT", [C, TB], bf16, kind="ExternalInput").ap()
    d_w = {}
    for name in ("wq", "wk", "wv", "cq", "ck", "cv"):
        d_w[name] = nc.dram_tensor(name, [L, P, NC, HD], bf16, kind="ExternalInput").ap()
    d_w1 = nc.dram_tensor("w1", [L, P, NC, FF], bf16, kind="ExternalInput").ap()
    d_w2 = nc.dram_tensor("w2", [L, P, NF, C], bf16, kind="ExternalInput").ap()
    d_bqk = nc.dram_tensor("bqk", [P, L * 16], f32, kind="ExternalInput").ap()
    d_bvb = nc.dram_tensor("bvb", [L, 2, P, HD], f32, kind="ExternalInput").ap()
    d_b1 = nc.dram_tensor("b1", [P, L * NF], f32, kind="ExternalInput").ap()
    d_b2 = nc.dram_tensor("b2", [P, L * NC], f32, kind="ExternalInput").ap()
    d_out = nc.dram_tensor("outT", [C, TB], f32, kind="ExternalOutput").ap()

    def act_recip(out, in_):
        # ACT-engine reciprocal (~1e-5 rel err measured on hw for this value
        # range); bass's wrapper refuses Reciprocal so emit directly.
        nc.scalar.add_instruction(mybir.InstActivation(
            name=nc.get_next_instruction_name(),
            func=AF.Reciprocal,
            ins=[nc.scalar.lower_ap(in_),
                 mybir.ImmediateValue(dtype=f32, value=0.0),
                 mybir.ImmediateValue(dtype=f32, value=1.0),
                 mybir.ImmediateValue(dtype=f32, value=0.0)],
            outs=[nc.scalar.lower_ap(out)],
        ))

    with tile.TileContext(nc) as tc:
        with ExitStack() as ctx:
            cpool = ctx.enter_context(tc.tile_pool(name="const", bufs=1))
            apool = ctx.enter_context(tc.tile_pool(name="acts", bufs=1))
            wpool = ctx.enter_context(tc.tile_pool(name="wts", bufs=1))
            wdpool = ctx.enter_context(tc.tile_pool(name="wdbl", bufs=2))
            tpool = ctx.enter_context(tc.tile_pool(name="tmp", bufs=2))
            ps_pj = ctx.enter_context(tc.tile_pool(name="pj", bufs=3, space="PSUM"))
            ps_sc = ctx.enter_context(tc.tile_pool(name="sc", bufs=3, space="PSUM"))
            ps_ao = ctx.enter_context(tc.tile_pool(name="ao", bufs=2, space="PSUM"))

            # ---- constants ----
            ones128b = cpool.tile([P, P], bf16, tag="ones128b")
            nc.vector.memset(ones128b, 1.0)
            onesN = cpool.tile([P, P], f32, tag="onesN")
            nc.vector.memset(onesN, NORM)      # ones/65536 for LN stats matmul
            eps_t = cpool.tile([P, 1], f32, tag="eps")
            nc.vector.memset(eps_t, EPS)
            zero_t = cpool.tile([P, 1], f32, tag="zero")
            nc.vector.memset(zero_t, 0.0)
            pre_t = cpool.tile([P, 1], f32, tag="pre")
            magic_t = cpool.tile([P, 4], dt.uint32, tag="magic")
            nc.vector.memset(magic_t, 0x1fbd1df5)  # sqrt-bits initial guess
            bqk_s = cpool.tile([P, L * 16], f32, tag="bqk")
            b1_s = cpool.tile([P, L * NF], f32, tag="b1")
            b2_s = cpool.tile([P, L * NC], f32, tag="b2")

            def act_preload(func):
                # dummy op so the activation-table load happens off-path
                if func == AF.Reciprocal:
                    act_recip(pre_t, eps_t)
                else:
                    nc.scalar.activation(pre_t, eps_t, func, bias=eps_t,
                                         scale=1.0)

            # ---- persistent activations (kc-major merged tiles per half) ----
            xres = [apool.tile([P, NC * 512], f32, tag=f"xres{h}", name=f"xres{h}")
                    for h in range(2)]
            xn = [apool.tile([P, NC * 512], bf16, tag=f"xn{h}", name=f"xn{h}")
                  for h in range(2)]
            eTs = [apool.tile([P, NC * 512], bf16, tag=f"eT{h}", name=f"eT{h}")
                   for h in range(2)]

            def load_io(dst_list, src, k, h):
                rs = slice(k * P, (k + 1) * P)
                cs_ = slice(h * 512, (h + 1) * 512)
                ts_ = slice(k * 512, (k + 1) * 512)
                nc.sync.dma_start(out=dst_list[h][:, ts_], in_=src[rs, cs_])

            def dma_w(wt, name, l):
                # chunked weight DMA: one dma_start per kc so transfers spread
                # across DMA queues (single queue is only ~20 GB/s)
                for kc in range(NC):
                    nc.sync.dma_start(out=wt[:, kc, :], in_=d_w[name][l][:, kc, :])

            def dma_w12(wt, dram, l, n2, n3, inner):
                chunk = n3 // inner
                for kc in range(n2):
                    for i in range(inner):
                        sl = slice(i * chunk, (i + 1) * chunk)
                        nc.sync.dma_start(out=wt[:, kc, sl], in_=dram[l][:, kc, sl])

            def new_wqkv():
                w = {}
                for name in ("wq", "wk", "wv"):
                    w[name] = wdpool.tile([P, NC, HD], bf16, tag=name, name=name)
                    dma_w(w[name], name, new_wqkv.layer)
                for i, name in enumerate(("bvs", "bvc")):
                    w[name] = wdpool.tile([P, HD], f32, tag=name, name=name)
                    nc.sync.dma_start(out=w[name], in_=d_bvb[new_wqkv.layer, i])
                return w

            # q/k/v weights for layer 0 + input activations, ordered so the
            # first projections can start as soon as possible
            new_wqkv.layer = 0
            wqkv_cur = new_wqkv()
            for k in range(NC):
                load_io(xn, d_xTb, k, 0)
            for k in range(NC):
                load_io(xn, d_xTb, k, 1)
            # bias tables are first needed ~20us in; keep them off the
            # startup-critical DMA queues
            nc.sync.dma_start(out=bqk_s, in_=d_bqk)
            nc.sync.dma_start(out=b1_s, in_=d_b1)
            nc.sync.dma_start(out=b2_s, in_=d_b2)
            for k in range(NC):
                for h in range(2):
                    load_io(xres, d_xT, k, h)
            for k in range(NC):
                for h in range(2):
                    load_io(eTs, d_eT, k, h)

            def xsl(kc, bb=None):
                if bb is None:
                    return slice(kc * 512, (kc + 1) * 512)
                return slice(kc * 512 + bb * P, kc * 512 + (bb + 1) * P)

            qT = [[apool.tile([P, 512], bf16, tag=f"qT{k}_{h}", name=f"qT{k}_{h}")
                   for h in range(2)] for k in range(NC)]
            kT = [[apool.tile([P, 512], bf16, tag=f"kT{k}_{h}", name=f"kT{k}_{h}")
                   for h in range(2)] for k in range(NC)]
            kcT = [[apool.tile([P, 512], bf16, tag=f"kcT{k}_{h}", name=f"kcT{k}_{h}")
                    for h in range(2)] for k in range(NC)]
            vS = [apool.tile([P, HD], bf16, tag=f"v{b}", name=f"v{b}") for b in range(BL)]
            vC = [apool.tile([P, HD], bf16, tag=f"vc{b}", name=f"vc{b}") for b in range(BL)]
            hT = [apool.tile([P, 512], bf16, tag=f"hT{f}", name=f"hT{f}") for f in range(NF)]

            def proj_qk(dst, wt, src, mi, l, scale, halves=(0, 1), evsplit=1):
                for h2 in halves:
                    for mc in range(NC):
                        pj = ps_pj.tile([P, 512], f32, tag="pj", name="pj")
                        for kc in range(NC):
                            nc.tensor.matmul(pj, wt[:, kc, mc * P:(mc + 1) * P],
                                             src[h2][:, xsl(kc)],
                                             start=(kc == 0), stop=(kc == NC - 1))
                        col = (l * 4 + mi) * 4 + mc
                        step = 512 // evsplit
                        for e in range(evsplit):
                            sl = slice(e * step, (e + 1) * step)
                            nc.scalar.activation(dst[mc][h2][:, sl], pj[:, sl],
                                                 AF.Identity,
                                                 bias=bqk_s[:, col:col + 1],
                                                 scale=scale)

            def proj_v(dst, wt, src, bias_tile, bs):
                for b in bs:
                    h2, bb = divmod(b, 4)
                    pj = ps_pj.tile([P, 512], f32, tag="pj", name="pj")
                    for kc in range(NC):
                        nc.tensor.matmul(pj, src[h2][:, xsl(kc, bb)],
                                         wt[:, kc, :],
                                         start=(kc == 0), stop=(kc == NC - 1))
                    nc.vector.tensor_tensor(dst[b], pj, bias_tile, op=OP.add)

            def new_st():
                return tpool.tile([P, 4, 5], f32, tag="ST", name="ST", bufs=8)

            def attention_half(kTl, vl, ST, h2, pe_filler=None, post_hook=None):
                # scores + exp per item; attn-out reads RAW exp scores and the
                # softmax 1/colsum is applied to the psum output at eviction,
                # so colsum/recip sit off the scores->attnout critical path.
                expTs = {}
                for bb in range(4):
                    sce = ps_sc.tile([P, 512], f32, tag="sc", name="sc")
                    sco = ps_sc.tile([P, 512], f32, tag="sc", name="sc")
                    for p in range(NPAIR):
                        nc.tensor.matmul(sce[:, p * P:(p + 1) * P],
                                         kTl[p][h2][0:64, bb * P:(bb + 1) * P],
                                         qT[p][h2][0:64, bb * P:(bb + 1) * P],
                                         start=True, stop=True,
                                         tile_position=(0, 0))
                        nc.tensor.matmul(sco[:, p * P:(p + 1) * P],
                                         kTl[p][h2][64:128, bb * P:(bb + 1) * P],
                                         qT[p][h2][64:128, bb * P:(bb + 1) * P],
                                         start=True, stop=True,
                                         tile_position=(64, 0))
                    expT = tpool.tile([P, TB], bf16, tag="expT", name="expT",
                                      bufs=4)
                    nc.scalar.activation(expT[:, 0:512], sce, AF.Exp,
                                         bias=zero_t, scale=1.0)
                    nc.scalar.activation(expT[:, 512:1024], sco, AF.Exp,
                                         bias=zero_t, scale=1.0)
                    expTs[bb] = expT
                # pull the Reciprocal table load off-path now; the Identity
                # evicts issued in fillers live in every table, so only
                # Exp<->Reciprocal transitions exist and both are preloaded
                act_preload(AF.Reciprocal)
                if pe_filler is not None:
                    pe_filler()
                x3 = xres[h2].rearrange("p (k n) -> p k n", k=NC)
                for bb in range(4):
                    b = h2 * 4 + bb
                    ao = ps_ao.tile([P, 512], f32, tag="ao", name="ao")
                    for p in range(NPAIR):
                        for j in range(2):
                            h = 2 * p + j
                            pos = (h % 2) * 512 + (h // 2) * P
                            nc.tensor.matmul(ao[j * 64:(j + 1) * 64, p * P:(p + 1) * P],
                                             vl[b][:, h * 64:(h + 1) * 64],
                                             expTs[bb][:, pos:pos + P],
                                             start=True, stop=True,
                                             tile_position=(0, j * 64))
                    # denominators for this item: colsum over s via M=64
                    # ones-matmuls into partition-halves of ONE psum tile
                    # (column-split tiles run concurrently on the PE), then a
                    # single full-width reciprocal: ddrC[j-rows] = 1/den_j
                    ddrC = tpool.tile([P, 512], bf16, tag="ddr", name="ddr",
                                      bufs=4)
                    dsum = ps_sc.tile([P, 512], f32, tag="sc", name="sc")
                    for j in range(2):
                        sl = slice(j * 512, (j + 1) * 512)
                        nc.tensor.matmul(dsum[j * 64:(j + 1) * 64, :],
                                         ones128b[:, 0:64], expTs[bb][:, sl],
                                         start=True, stop=True,
                                         tile_position=(0, j * 64))
                    act_recip(ddrC, dsum)
                    aot = tpool.tile([P, 512], f32, tag="aot", name="aot",
                                     bufs=4)
                    nc.vector.tensor_tensor(aot, ao, ddrC, op=OP.mult)
                    dst = x3[:, :, bb * P:(bb + 1) * P]
                    nc.vector.scalar_tensor_tensor(
                        dst, aot.rearrange("p (k n) -> p k n", k=NC),
                        0.0, dst, op0=OP.add, op1=OP.add,
                        accum_out=ST[:, bb, 0:1])
                if post_hook is not None:
                    post_hook()

            def rsqrt_dve(rr, v):
                # rr = 1/sqrt(v) DVE-only (no ACT table): accurate reciprocal,
                # sqrt-bits initial guess, 2 Newton steps (~5e-6 rel err)
                r = tpool.tile([P, 4], f32, tag="rs_r", name="rs_r")
                nc.vector.reciprocal(r, v)
                yu = rr.bitcast(dt.uint32)
                nc.vector.tensor_scalar(yu, r.bitcast(dt.uint32), 1, None,
                                        op0=OP.logical_shift_right)
                nc.vector.tensor_tensor(yu, yu, magic_t, op=OP.add)
                t = tpool.tile([P, 4], f32, tag="rs_t", name="rs_t")
                for _ in range(2):
                    nc.vector.tensor_tensor(t, rr, rr, op=OP.mult)
                    nc.vector.tensor_tensor(t, t, v, op=OP.mult)
                    nc.vector.tensor_scalar(t, t, -0.5, 1.5,
                                            op0=OP.mult, op1=OP.add)
                    nc.vector.tensor_tensor(rr, rr, t, op=OP.mult)

            def ln_sq(ST, h2, nsum=1):
                # stats per batch item over (T, C); col 0(-3) summed at evict,
                # one fused square+reduce per item fills col 4. Split across
                # ACT (Square lives in every table) and DVE to halve the
                # serial chain length.
                x3 = xres[h2].rearrange("p (k n) -> p k n", k=NC)
                for bb in range(4):
                    sq = tpool.tile([P, 512], bf16, tag="sq", name="sq")
                    src = x3[:, :, bb * P:(bb + 1) * P]
                    nc.vector.scalar_tensor_tensor(
                        sq.rearrange("p (k n) -> p k n", k=NC), src, 1.0,
                        src, op0=OP.mult, op1=OP.mult,
                        accum_out=ST[:, bb, 4:5])
                if nsum == 1:
                    # cols 1-3 unwritten in attention mode; zero them so the
                    # stats matmul never reads uninitialized sbuf
                    nc.vector.memset(ST[:, :, 1:4], 0.0)

            def ln_fin(ST, h2, nsum=1, last=False, dma_out=False):
                x3 = xres[h2].rearrange("p (k n) -> p k n", k=NC)
                tot = ps_pj.tile([P, 20], f32, tag="pj", name="pj")
                nc.tensor.matmul(tot, onesN,
                                 ST.rearrange("p a b -> p (a b)"),
                                 start=True, stop=True)
                tot3 = tot.rearrange("p (a b) -> p a b", b=5)
                mm_ = tpool.tile([P, 4], f32, tag="mm_", name="mm_")
                if nsum == 1:
                    nc.vector.tensor_copy(out=mm_, in_=tot3[:, :, 0])
                else:
                    nc.vector.reduce_sum(mm_, tot3[:, :, 0:nsum],
                                         axis=mybir.AxisListType.X)
                var = tpool.tile([P, 4], f32, tag="var", name="var")
                nc.vector.tensor_tensor(var, mm_, mm_, op=OP.mult)
                # var = (sqsum + eps) - mu^2, eps folded in
                nc.vector.scalar_tensor_tensor(var, tot3[:, :, 4], eps_t, var,
                                               op0=OP.add, op1=OP.subtract)
                rr = tpool.tile([P, 4], f32, tag="rr", name="rr")
                rsqrt_dve(rr, var)
                xn3 = xn[h2].rearrange("p (k n) -> p k n", k=NC)
                for bb in range(4):
                    src = x3[:, :, bb * P:(bb + 1) * P]
                    if not last:
                        # bf16 normalized copy first — unblocks the next
                        # phase's matmuls before the fp32 in-place update
                        nc.vector.tensor_scalar(xn3[:, :, bb * P:(bb + 1) * P],
                                                src, mm_[:, bb:bb + 1],
                                                rr[:, bb:bb + 1],
                                                op0=OP.subtract, op1=OP.mult)
                for bb in range(4):
                    src = x3[:, :, bb * P:(bb + 1) * P]
                    nc.vector.tensor_scalar(src, src, mm_[:, bb:bb + 1],
                                            rr[:, bb:bb + 1],
                                            op0=OP.subtract, op1=OP.mult)
                    if dma_out:
                        for k in range(NC):
                            nc.sync.dma_start(
                                out=d_out[k * P:(k + 1) * P,
                                          h2 * 512 + bb * P:
                                          h2 * 512 + (bb + 1) * P],
                                in_=xres[h2][:, k * 512 + bb * P:
                                             k * 512 + (bb + 1) * P])

            def ln_issue(ST, h2, nsum=1, last=False, dma_out=False):
                ln_sq(ST, h2, nsum)
                ln_fin(ST, h2, nsum, last, dma_out)

            def ffn_w1(h2, w1s, l):
                for fc in range(NF):
                    pj = ps_pj.tile([P, 512], f32, tag="pj", name="pj")
                    for kc in range(NC):
                        nc.tensor.matmul(pj, w1s[:, kc, fc * P:(fc + 1) * P],
                                         xn[h2][:, xsl(kc)],
                                         start=(kc == 0), stop=(kc == NC - 1))
                    col = l * NF + fc
                    nc.scalar.activation(hT[fc], pj, AF.Relu,
                                         bias=b1_s[:, col:col + 1], scale=1.0)

            def ffn_w2(h2, ST, w2s, l, mcs=range(NC)):
                for mc in mcs:
                    pj = ps_pj.tile([P, 512], f32, tag="pj", name="pj")
                    for fc in range(NF):
                        nc.tensor.matmul(pj, w2s[:, fc, mc * P:(mc + 1) * P],
                                         hT[fc],
                                         start=(fc == 0), stop=(fc == NF - 1))
                    b2col = b2_s[:, l * NC + mc:l * NC + mc + 1]
                    for bb in range(4):
                        dst = xres[h2][:, xsl(mc, bb)]
                        nc.vector.scalar_tensor_tensor(
                            dst, pj[:, bb * P:(bb + 1) * P],
                            b2col, dst, op0=OP.add, op1=OP.add,
                            accum_out=ST[:, bb, mc:mc + 1])

            for l in range(L):
                # single-buffered weights for this layer (all consumed
                # mid-layer, so their DMA overlaps the previous layer's tail)
                bvs = wqkv_cur["bvs"]
                bvc = wqkv_cur["bvc"]
                wts = {}
                for name in ("cq", "ck", "cv"):
                    w = wpool.tile([P, NC, HD], bf16, tag=name, name=name)
                    dma_w(w, name, l)
                    wts[name] = w
                w1s = wpool.tile([P, NC, FF], bf16, tag="w1", name="w1")
                dma_w12(w1s, d_w1, l, NC, FF, 4)
                w2s = wpool.tile([P, NF, C], bf16, tag="w2", name="w2")
                dma_w12(w2s, d_w2, l, NF, C, 1)
                # prefetch next layer's q/k/v weights (double-buffered pool)
                if l + 1 < L:
                    new_wqkv.layer = l + 1
                    wqkv_next = new_wqkv()

                # --- SA projections: h0 q/k came from the previous layer's
                # tail (except l=0); v and h1 parts issue here ---
                if l == 0:
                    proj_qk(qT, wqkv_cur["wq"], xn, 0, l, 0.125, halves=(0,))
                    proj_qk(kT, wqkv_cur["wk"], xn, 1, l, 1.0, halves=(0,))
                proj_v(vS, wqkv_cur["wv"], xn, bvs, range(0, 4))
                proj_qk(qT, wqkv_cur["wq"], xn, 0, l, 0.125, halves=(1,))
                proj_qk(kT, wqkv_cur["wk"], xn, 1, l, 1.0, halves=(1,))
                proj_v(vS, wqkv_cur["wv"], xn, bvs, range(4, 8))

                ST1 = [new_st(), new_st()]
                ST2 = [new_st(), new_st()]
                ST3 = [new_st(), new_st()]

                # --- self attention ---
                def fill_sa0():
                    proj_qk(kcT, wts["ck"], eTs, 3, l, 1.0, halves=(0,))
                    proj_v(vC, wts["cv"], eTs, bvc, range(0, 4))

                attention_half(kT, vS, ST1[0], 0, pe_filler=fill_sa0)

                def fill_sa1():
                    # LN1h0 entirely ahead of the cv evicts in the DVE queue;
                    # its stats matmul is covered by ck-h1's projections
                    ln_sq(ST1[0], 0)
                    proj_qk(kcT, wts["ck"], eTs, 3, l, 1.0, halves=(1,))
                    ln_fin(ST1[0], 0)
                    proj_v(vC, wts["cv"], eTs, bvc, range(4, 8))

                def post_sa1():
                    proj_qk(qT, wts["cq"], xn, 2, l, 0.125, halves=(0,),
                            evsplit=2)
                    act_preload(AF.Exp)

                attention_half(kT, vS, ST1[1], 1,
                               pe_filler=fill_sa1, post_hook=post_sa1)

                # --- cross attention ---
                def fill_ca0():
                    ln_issue(ST1[1], 1)

                def post_ca0():
                    proj_qk(qT, wts["cq"], xn, 2, l, 0.125, halves=(1,),
                            evsplit=2)
                    ln_issue(ST2[0], 0)
                    act_preload(AF.Exp)

                attention_half(kcT, vC, ST2[0], 0,
                               pe_filler=fill_ca0, post_hook=post_ca0)

                def fill_ca1():
                    # FFN h0 fills CA1's ACT-bound window (xn[0] ready: LN2h0
                    # was issued in post_ca0)
                    ffn_w1(0, w1s, l)

                attention_half(kcT, vC, ST2[1], 1, pe_filler=fill_ca1)

                # --- feed-forward ---
                ln_sq(ST2[1], 1)
                ffn_w2(0, ST3[0], w2s, l, mcs=(0,))
                ln_fin(ST2[1], 1)
                ffn_w2(0, ST3[0], w2s, l, mcs=(1, 2, 3))
                ffn_w1(1, w1s, l)
                ln_issue(ST3[0], 0, nsum=4, last=(l == L - 1),
                         dma_out=(l == L - 1))
                ffn_w2(1, ST3[1], w2s, l)

                # --- layer tail: pipeline next layer's h0 q/k projections
                # around the final LN of this layer ---
                if l + 1 < L:
                    proj_qk(qT, wqkv_next["wq"], xn, 0, l + 1, 0.125,
                            halves=(0,))
                    ln_issue(ST3[1], 1, nsum=4)
                    proj_qk(kT, wqkv_next["wk"], xn, 1, l + 1, 1.0,
                            halves=(0,))
                    wqkv_cur = wqkv_next
                else:
                    ln_issue(ST3[1], 1, nsum=4, last=True, dma_out=True)

    nc.compile()
    return nc


def _prep_shared(inputs):
    """Host-side weight repacking (shared across cores)."""
    def packw(w):  # [L,H,C,DH] -> [L,128,NC,H*DH]  (c = kc*128+p)
        w2 = np.ascontiguousarray(w.transpose(0, 2, 1, 3)).reshape(L, C, HD)
        return np.ascontiguousarray(
            w2.reshape(L, NC, P, HD).transpose(0, 2, 1, 3)).astype(_BF)

    shared = {}
    for nm, key in (("wq", "sa_wq"), ("wk", "sa_wk"), ("wv", "sa_wv"),
                    ("cq", "ca_wq"), ("ck", "ca_wk"), ("cv", "ca_wv")):
        shared[nm] = packw(inputs[key])
    shared["w1"] = np.ascontiguousarray(
        inputs["ff_w1"].reshape(L, NC, P, FF).transpose(0, 2, 1, 3)).astype(_BF)
    shared["w2"] = np.ascontiguousarray(
        inputs["ff_w2"].reshape(L, NF, P, C).transpose(0, 2, 1, 3)).astype(_BF)

    bqk = np.zeros((P, L * 16), np.float32)
    for l in range(L):
        for mi, (bias, s) in enumerate((
                (inputs["sa_bq"][l], 0.125), (inputs["sa_bk"][l], 1.0),
                (inputs["ca_bq"][l], 0.125), (inputs["ca_bk"][l], 1.0))):
            flat = bias.reshape(HD).astype(np.float32) * s
            for mc in range(NC):
                bqk[:, (l * 4 + mi) * 4 + mc] = flat[mc * P:(mc + 1) * P]
    shared["bqk"] = bqk

    bv = np.stack([inputs["sa_bv"].reshape(L, HD),
                   inputs["ca_bv"].reshape(L, HD)], axis=1).astype(np.float32)
    shared["bvb"] = np.ascontiguousarray(
        np.broadcast_to(bv[:, :, None, :], (L, 2, P, HD)))

    b1 = np.zeros((P, L * NF), np.float32)
    for l in range(L):
        for fc in range(NF):
            b1[:, l * NF + fc] = inputs["ff_b1"][l, fc * P:(fc + 1) * P]
    shared["b1"] = b1
    b2 = np.zeros((P, L * NC), np.float32)
    for l in range(L):
        for mc in range(NC):
            b2[:, l * NC + mc] = inputs["ff_b2"][l, mc * P:(mc + 1) * P]
    shared["b2"] = b2
    return shared


LAST_RESULT = None


def _install_ntff_hook():
    """Register the axon NTFF profile hook that the image's antenv lacks.

    Only used for local benchmarking (KERNEL_TRACE=1); inert otherwise.
    """
    import sys
    import types
    try:
        import antenv
        if getattr(antenv, "axon_hooks", None) is not None:
            return
        from trn_agent_boot.trn_boot import _ntff_profile_via_ctypes
        mod = types.ModuleType("antenv.axon_hooks")
        mod._hook = _ntff_profile_via_ctypes("/opt/axon/libaxon_pjrt.so")

        def get_axon_ntff_profile_hook():
            return mod._hook

        def set_axon_ntff_profile_hook(h):
            mod._hook = h

        mod.get_axon_ntff_profile_hook = get_axon_ntff_profile_hook
        mod.set_axon_ntff_profile_hook = set_axon_ntff_profile_hook
        sys.modules["antenv.axon_hooks"] = mod
        antenv.axon_hooks = mod
    except Exception as e:  # pragma: no cover - profiling is best-effort
        print(f"ntff hook install failed: {e}")


def kernel(**inputs):
    global LAST_RESULT
    import os
    inputs = {k: np.asarray(v) for k, v in inputs.items()}
    if "nc" not in _cache:
        _cache["nc"] = _build()
    nc = _cache["nc"]

    shared = _prep_shared(inputs)
    x = inputs["x"].astype(np.float32)
    enc = inputs["encoder_output"].astype(np.float32)

    in_maps = []
    for core in range(NCORES):
        sl = slice(core * BL, (core + 1) * BL)
        xT = np.ascontiguousarray(x[sl].transpose(2, 0, 1)).reshape(C, TB)
        eT = np.ascontiguousarray(enc[sl].transpose(2, 0, 1)).reshape(C, TB)
        m = dict(shared)
        m["xT"] = xT
        m["xTb"] = xT.astype(_BF)
        m["eT"] = eT.astype(_BF)
        in_maps.append(m)

    trace = bool(int(os.environ.get("KERNEL_TRACE", "0")))
    if trace:
        _install_ntff_hook()
    from concourse.bass_utils import run_bass_kernel_spmd
    res = run_bass_kernel_spmd(nc, in_maps, list(range(NCORES)), trace=trace,
                               trace_cores=[0])
    LAST_RESULT = res

    out = np.empty((B, T, C), np.float32)
    for core in range(NCORES):
        outT = res.results[core]["outT"]  # [C, TB]
        out[core * BL:(core + 1) * BL] = outT.reshape(C, BL, T).transpose(1, 2, 0)
    return out
